# revision 30
# baseline (speedup 1.0000x reference)
"""Trainium2 Bass kernel for the DF time-loop module (nn_DfOpTimeLoop).

Strategy
--------
Shard the T=60000 time axis across 8 NeuronCores (7500 frames each, padded
to 7680 = 128*60 on-device so tiles use 128 partitions — a multiple of 16
so DMA descriptors spray all 16 SDMA engines). All the reference's quirky
edge behavior folds into a host-built halo buffer H (frames 0/1 swapped,
zero rows prepended/appended), and the alpha blend + passthrough-base folds
into host-built planar coefficient tensors, so each core runs a uniform
5-tap sliding-window complex MAC with zero epilogue:

  H = [0, 0, spec[1], spec[0], spec[2], ..., spec[T-1], 0, 0, ...]
  d_e[t,j,f] = alpha[t]*cre[t,j,f] + (1-alpha[t])*delta(j==2)
  d_o[t,j,f] = -alpha[t]*cim[t,j,f]

  per-core (local t): wine[t,j,f] = se[t+j, f], wino[t,j,f] = so[t+j, f]
    o[t, 2f]   = sum_j wine*d_e + wino*d_o
    o[t, 2f+1] = sum_j wino*d_e - wine*d_o
    o[t, 192:] = s32[t+2, :]            (pure DRAM->DRAM copy)

The DF window (se/so) and coefs (d_e/d_o) ship as de-interleaved (planar)
bf16 arrays: every device product is then a fully contiguous bf16
tensor_tensor (DVE 2x mode) and every load a multi-KB contiguous DMA
descriptor. Sums are f32 (end-to-end L2 rel err ~2e-3). The passthrough
columns stay f32 and never touch compute.

On-core tiling: one resident tile of 128 partitions x 60 frames/partition
(the whole 7680-frame shard; no inter-tile reload stalls); the s loads
have a 4-row per-partition overlap so all 5 taps are free-dim shifts,
and compute streams in 12 chunks of 5 frames with per-chunk stores.
All compute runs on DVE (GpSimd SBUF-port contention poisons concurrent
DVE ops ~3x, so it stays idle); the j-reduction is a strided-read
tensor_reduce whose strided interleaved-output write is hidden behind the
5:1 read:write ratio. Sync queue: loads; Scalar queue: DRAM->DRAM
passthrough + stores (its in-order parking blocks nothing).
"""

import numpy as np

NFREQ = 481
NDF = 96
ORDER = 5
W = 2 * NFREQ          # 962 floats per output/spec row
C = 2 * NDF            # 192 DF values per row
PW = W - C             # 770 passthrough values per row
JF = ORDER * NDF       # 480 planar coef values per frame

N_CORES = 8
T_FULL = 60000
TC = T_FULL // N_CORES         # real frames per core
TC_PAD = 7680                  # = 128 * 60, padded on-device frame count

P_DIM = 128
U_FR = 60
UC = 5
PASS_SPLIT = 6

_NC_CACHE = {}


def _build_nc():
    import concourse.bass as bass
    import concourse.bacc as bacc
    import concourse.mybir as mybir
    from concourse.mybir import AluOpType
    from concourse.tile import TileContext

    F32 = mybir.dt.float32
    BF16 = mybir.dt.bfloat16
    Tc, P, U = TC_PAD, P_DIM, U_FR
    N = P * U
    ntiles = Tc // N
    assert ntiles * N == Tc
    M = UC * JF

    def _view(ap, off, dims):
        return bass.AP(ap.tensor, ap.offset + off, [list(d) for d in dims])

    def _tview(t_ap, off, dims):
        return bass.AP(
            t_ap.tensor, t_ap.offset + off,
            [list(t_ap.ap[0])] + [list(d) for d in dims],
        )

    nc = bacc.Bacc("TRN2", target_bir_lowering=False, debug=False)
    SE = nc.dram_tensor("se", [Tc + 4, NDF], BF16, kind="ExternalInput").ap()
    SO = nc.dram_tensor("so", [Tc + 4, NDF], BF16, kind="ExternalInput").ap()
    S32 = nc.dram_tensor("s32", [Tc + 4, PW], F32, kind="ExternalInput").ap()
    DE = nc.dram_tensor("de", [Tc, JF], BF16, kind="ExternalInput").ap()
    DO = nc.dram_tensor("do", [Tc, JF], BF16, kind="ExternalInput").ap()
    O = nc.dram_tensor("o", [Tc, W], F32, kind="ExternalOutput").ap()

    with TileContext(nc) as tc:
        with (
            tc.tile_pool(name="sp", bufs=1) as sp,
            tc.tile_pool(name="dp", bufs=8) as dp,
            tc.tile_pool(name="op_", bufs=2) as op_,
            tc.tile_pool(name="wp", bufs=2) as wp,
        ):
            for it in range(ntiles):
                base = it * N

                se_t = sp.tile([P, (U + 4) * NDF], BF16, tag="se")
                so_t = sp.tile([P, (U + 4) * NDF], BF16, tag="so")
                nc.sync.dma_start(
                    out=_tview(se_t, 0, [(1, (U + 4) * NDF)]),
                    in_=_view(
                        SE, base * NDF, [(U * NDF, P), (1, (U + 4) * NDF)]
                    ),
                )
                # so rides the scalar queue: both 1.6MB s-transfers then
                # drain in parallel, halving the cold-start load latency.
                nc.scalar.dma_start(
                    out=_tview(so_t, 0, [(1, (U + 4) * NDF)]),
                    in_=_view(
                        SO, base * NDF, [(U * NDF, P), (1, (U + 4) * NDF)]
                    ),
                )

                for uc0 in range(0, U, UC):
                    de_t = dp.tile([P, UC * JF], BF16, tag="de")
                    do_t = dp.tile([P, UC * JF], BF16, tag="do")
                    nc.sync.dma_start(
                        out=_tview(de_t, 0, [(1, UC * JF)]),
                        in_=_view(
                            DE, (base + uc0) * JF, [(U * JF, P), (1, UC * JF)]
                        ),
                    )
                    nc.sync.dma_start(
                        out=_tview(do_t, 0, [(1, UC * JF)]),
                        in_=_view(
                            DO, (base + uc0) * JF, [(U * JF, P), (1, UC * JF)]
                        ),
                    )

                    o_t = op_.tile([P, UC * C], F32, tag="o", bufs=4)

                    wine = _tview(
                        se_t, uc0 * NDF, [(NDF, UC), (NDF, ORDER), (1, NDF)]
                    )
                    wino = _tview(
                        so_t, uc0 * NDF, [(NDF, UC), (NDF, ORDER), (1, NDF)]
                    )
                    d_flat = [(1, UC * JF)]

                    A = wp.tile([P, M], BF16, tag="A", bufs=1)      # wine*de
                    B = wp.tile([P, M], BF16, tag="B", bufs=1)      # wino*do
                    Cc = wp.tile([P, M], BF16, tag="Cc", bufs=1)    # wino*de
                    Dd = wp.tile([P, M], BF16, tag="Dd", bufs=1)    # wine*do
                    nc.vector.tensor_tensor(
                        A[:], wine, _tview(de_t, 0, d_flat), AluOpType.mult)
                    nc.vector.tensor_tensor(
                        B[:], wino, _tview(do_t, 0, d_flat), AluOpType.mult)
                    nc.vector.tensor_tensor(
                        Cc[:], wino, _tview(de_t, 0, d_flat), AluOpType.mult)
                    nc.vector.tensor_tensor(
                        Dd[:], wine, _tview(do_t, 0, d_flat), AluOpType.mult)

                    # E (real path) and Mm (imag path) share one tile so
                    # the j-tree and final reduce process both paths per op.
                    EM = wp.tile([P, 2 * M], BF16, tag="EM", bufs=3)
                    nc.vector.tensor_tensor(
                        _tview(EM, 0, [(1, M)]), A[:], B[:], AluOpType.add)
                    nc.vector.tensor_tensor(
                        _tview(EM, M, [(1, M)]), Cc[:], Dd[:],
                        AluOpType.subtract)

                    # j-tree: fold the 5 taps into 2 stacked halves with
                    # contiguous adds (both paths per op), then one 2-slot
                    # strided reduce into the interleaved o_t.
                    VF = UC * NDF

                    def js2(j):
                        return _tview(
                            EM, j * NDF, [(M, 2), (JF, UC), (1, NDF)]
                        )

                    Z2 = wp.tile([P, 4 * VF], BF16, tag="Z2", bufs=3)
                    zt2 = wp.tile([P, 2 * VF], BF16, tag="zt2", bufs=1)
                    nc.vector.tensor_tensor(
                        _tview(Z2, 0, [(2 * VF, 2), (NDF, UC), (1, NDF)]),
                        js2(0), js2(1), AluOpType.add)
                    nc.vector.tensor_tensor(
                        _tview(zt2, 0, [(VF, 2), (NDF, UC), (1, NDF)]),
                        js2(2), js2(3), AluOpType.add)
                    nc.vector.tensor_tensor(
                        _tview(Z2, VF, [(2 * VF, 2), (NDF, UC), (1, NDF)]),
                        _tview(zt2, 0, [(VF, 2), (NDF, UC), (1, NDF)]),
                        js2(4), AluOpType.add)
                    nc.vector.tensor_reduce(
                        out=_tview(o_t, 0, [(1, 2), (C, UC), (2, NDF)]),
                        in_=_tview(
                            Z2, 0,
                            [(2 * VF, 2), (NDF, UC), (1, NDF), (VF, 2)],
                        ),
                        axis=mybir.AxisListType.X,
                        op=AluOpType.add,
                    )

                    nc.scalar.dma_start(
                        out=_view(
                            O, (base + uc0) * W, [(U * W, P), (W, UC), (1, C)]
                        ),
                        in_=_tview(o_t, 0, [(C, UC), (1, C)]),
                    )

                rows_per = N // PASS_SPLIT
                for ps in range(PASS_SPLIT):
                    r0 = base + ps * rows_per
                    nc.gpsimd.dma_start(
                        out=_view(O, r0 * W + C, [(W, rows_per), (1, PW)]),
                        in_=_view(
                            S32, (r0 + 2) * PW, [(PW, rows_per), (1, PW)]
                        ),
                    )

    nc.compile()
    return nc


def get_nc():
    if "nc" not in _NC_CACHE:
        _NC_CACHE["nc"] = _build_nc()
    return _NC_CACHE["nc"]


def prepare_inputs(spec, coefs, alpha):
    """Host-side shard prep. Returns in_maps for the 8 cores."""
    import ml_dtypes

    bf16 = ml_dtypes.bfloat16
    spec = np.ascontiguousarray(spec, dtype=np.float32)
    coefs = np.ascontiguousarray(coefs, dtype=np.float32)
    alpha = np.ascontiguousarray(alpha, dtype=np.float32)
    T = spec.shape[0]
    assert T == T_FULL

    h_rows = (N_CORES - 1) * TC + TC_PAD + 4
    # swapped-halo DF planes (bf16) and passthrough plane (f32)
    HE = np.zeros((h_rows, NDF), bf16)
    HO = np.zeros((h_rows, NDF), bf16)
    HP = np.zeros((h_rows, PW), np.float32)
    sw = np.arange(T)
    sw[0], sw[1] = 1, 0
    HE[2 : T + 2] = spec[sw, :NDF, 0].astype(bf16)
    HO[2 : T + 2] = spec[sw, :NDF, 1].astype(bf16)
    HP[2 : T + 2] = spec[sw, NDF:, :].reshape(T, PW)

    d_rows = (N_CORES - 1) * TC + TC_PAD
    a = np.ascontiguousarray(alpha, dtype=np.float32)[:, 0, None, None]
    DEv = np.zeros((d_rows, ORDER, NDF), np.float32)
    DOv = np.zeros((d_rows, ORDER, NDF), np.float32)
    np.multiply(a, coefs[..., 0], out=DEv[:T])
    np.multiply(-a, coefs[..., 1], out=DOv[:T])
    DEv[:T, 2, :] += (1.0 - a[:, 0, 0])[:, None]  # base tap: win[t,2] = H[t+2]
    DEv = DEv.reshape(d_rows, JF).astype(bf16)
    DOv = DOv.reshape(d_rows, JF).astype(bf16)

    in_maps = [
        {
            "se": HE[c * TC : c * TC + TC_PAD + 4],
            "so": HO[c * TC : c * TC + TC_PAD + 4],
            "s32": HP[c * TC : c * TC + TC_PAD + 4],
            "de": DEv[c * TC : c * TC + TC_PAD],
            "do": DOv[c * TC : c * TC + TC_PAD],
        }
        for c in range(N_CORES)
    ]
    return in_maps


def run_spmd(in_maps, trace=False, **kwargs):
    from concourse.bass_utils import run_bass_kernel_spmd

    nc = get_nc()
    return run_bass_kernel_spmd(
        nc, in_maps, list(range(N_CORES)), trace=trace, **kwargs
    )


def kernel(spec, coefs, alpha):
    in_maps = prepare_inputs(spec, coefs, alpha)
    res = run_spmd(in_maps).results
    out = np.concatenate([r["o"][:TC] for r in res], axis=0)
    return out.reshape(T_FULL, NFREQ, 2)


# revision 32
# speedup vs baseline: 1.0095x; 1.0095x over previous
"""Trainium2 Bass kernel for the DF time-loop module (nn_DfOpTimeLoop).

Strategy
--------
Shard the T=60000 time axis across 8 NeuronCores (7500 frames each, padded
to 7680 = 128*60 on-device so tiles use 128 partitions — a multiple of 16
so DMA descriptors spray all 16 SDMA engines). All the reference's quirky
edge behavior folds into a host-built halo buffer H (frames 0/1 swapped,
zero rows prepended/appended), and the alpha blend + passthrough-base folds
into host-built planar coefficient tensors, so each core runs a uniform
5-tap sliding-window complex MAC with zero epilogue:

  H = [0, 0, spec[1], spec[0], spec[2], ..., spec[T-1], 0, 0, ...]
  d_e[t,j,f] = alpha[t]*cre[t,j,f] + (1-alpha[t])*delta(j==2)
  d_o[t,j,f] = -alpha[t]*cim[t,j,f]

  per-core (local t): wine[t,j,f] = se[t+j, f], wino[t,j,f] = so[t+j, f]
    o[t, 2f]   = sum_j wine*d_e + wino*d_o
    o[t, 2f+1] = sum_j wino*d_e - wine*d_o
    o[t, 192:] = s32[t+2, :]            (pure DRAM->DRAM copy)

The DF window (se/so) and coefs (d_e/d_o) ship as de-interleaved (planar)
bf16 arrays: every device product is then a fully contiguous bf16
tensor_tensor (DVE 2x mode) and every load a multi-KB contiguous DMA
descriptor. Sums are f32 (end-to-end L2 rel err ~2e-3). The passthrough
columns stay f32 and never touch compute.

On-core tiling: one resident tile of 128 partitions x 60 frames/partition
(the whole 7680-frame shard; no inter-tile reload stalls); the s loads
have a 4-row per-partition overlap so all 5 taps are free-dim shifts,
and compute streams in 12 chunks of 5 frames with per-chunk stores.
All compute runs on DVE (GpSimd SBUF-port contention poisons concurrent
DVE ops ~3x, so it stays idle); the j-reduction is a strided-read
tensor_reduce whose strided interleaved-output write is hidden behind the
5:1 read:write ratio. Sync queue: loads; Scalar queue: DRAM->DRAM
passthrough + stores (its in-order parking blocks nothing).
"""

import numpy as np

NFREQ = 481
NDF = 96
ORDER = 5
W = 2 * NFREQ          # 962 floats per output/spec row
C = 2 * NDF            # 192 DF values per row
PW = W - C             # 770 passthrough values per row
JF = ORDER * NDF       # 480 planar coef values per frame

N_CORES = 8
T_FULL = 60000
TC = T_FULL // N_CORES         # real frames per core
TC_PAD = 7680                  # = 128 * 60, padded on-device frame count

P_DIM = 128
U_FR = 60
UC = 5
PASS_SPLIT = 6

_NC_CACHE = {}


def _build_nc():
    import concourse.bass as bass
    import concourse.bacc as bacc
    import concourse.mybir as mybir
    from concourse.mybir import AluOpType
    from concourse.tile import TileContext

    F32 = mybir.dt.float32
    BF16 = mybir.dt.bfloat16
    Tc, P, U = TC_PAD, P_DIM, U_FR
    N = P * U
    ntiles = Tc // N
    assert ntiles * N == Tc
    M = UC * JF

    def _view(ap, off, dims):
        return bass.AP(ap.tensor, ap.offset + off, [list(d) for d in dims])

    def _tview(t_ap, off, dims):
        return bass.AP(
            t_ap.tensor, t_ap.offset + off,
            [list(t_ap.ap[0])] + [list(d) for d in dims],
        )

    nc = bacc.Bacc("TRN2", target_bir_lowering=False, debug=False)
    SE = nc.dram_tensor("se", [Tc + 4, NDF], BF16, kind="ExternalInput").ap()
    SO = nc.dram_tensor("so", [Tc + 4, NDF], BF16, kind="ExternalInput").ap()
    S32 = nc.dram_tensor("s32", [Tc + 4, PW], F32, kind="ExternalInput").ap()
    DE = nc.dram_tensor("de", [Tc, JF], BF16, kind="ExternalInput").ap()
    DO = nc.dram_tensor("do", [Tc, JF], BF16, kind="ExternalInput").ap()
    O = nc.dram_tensor("o", [Tc, W], F32, kind="ExternalOutput").ap()

    with TileContext(nc) as tc:
        with (
            tc.tile_pool(name="sp", bufs=1) as sp,
            tc.tile_pool(name="dp", bufs=6) as dp,
            tc.tile_pool(name="op_", bufs=2) as op_,
            tc.tile_pool(name="wp", bufs=2) as wp,
        ):
            for it in range(ntiles):
                base = it * N

                se_t = sp.tile([P, (U + 4) * NDF], BF16, tag="se")
                so_t = sp.tile([P, (U + 4) * NDF], BF16, tag="so")
                # s loads split in two segments (rows 0:20 then 20:64) so
                # the first chunks' products wait only on the small head
                # segment; so rides the scalar queue so both planes drain
                # in parallel at cold start.
                SEG1 = 20
                for r0, r1 in ((0, SEG1), (SEG1, U + 4)):
                    nc.sync.dma_start(
                        out=_tview(se_t, r0 * NDF, [(1, (r1 - r0) * NDF)]),
                        in_=_view(
                            SE, (base + r0) * NDF,
                            [(U * NDF, P), (1, (r1 - r0) * NDF)],
                        ),
                    )
                    nc.scalar.dma_start(
                        out=_tview(so_t, r0 * NDF, [(1, (r1 - r0) * NDF)]),
                        in_=_view(
                            SO, (base + r0) * NDF,
                            [(U * NDF, P), (1, (r1 - r0) * NDF)],
                        ),
                    )

                for uc0 in range(0, U, UC):
                    de_t = dp.tile([P, UC * JF], BF16, tag="de")
                    do_t = dp.tile([P, UC * JF], BF16, tag="do")
                    nc.sync.dma_start(
                        out=_tview(de_t, 0, [(1, UC * JF)]),
                        in_=_view(
                            DE, (base + uc0) * JF, [(U * JF, P), (1, UC * JF)]
                        ),
                    )
                    nc.sync.dma_start(
                        out=_tview(do_t, 0, [(1, UC * JF)]),
                        in_=_view(
                            DO, (base + uc0) * JF, [(U * JF, P), (1, UC * JF)]
                        ),
                    )

                    o_t = op_.tile([P, UC * C], F32, tag="o", bufs=4)

                    wine = _tview(
                        se_t, uc0 * NDF, [(NDF, UC), (NDF, ORDER), (1, NDF)]
                    )
                    wino = _tview(
                        so_t, uc0 * NDF, [(NDF, UC), (NDF, ORDER), (1, NDF)]
                    )
                    d_flat = [(1, UC * JF)]

                    A = wp.tile([P, M], BF16, tag="A")      # wine*de
                    B = wp.tile([P, M], BF16, tag="B")      # wino*do
                    Cc = wp.tile([P, M], BF16, tag="Cc")    # wino*de
                    Dd = wp.tile([P, M], BF16, tag="Dd")    # wine*do
                    nc.vector.tensor_tensor(
                        A[:], wine, _tview(de_t, 0, d_flat), AluOpType.mult)
                    nc.vector.tensor_tensor(
                        B[:], wino, _tview(do_t, 0, d_flat), AluOpType.mult)
                    nc.vector.tensor_tensor(
                        Cc[:], wino, _tview(de_t, 0, d_flat), AluOpType.mult)
                    nc.vector.tensor_tensor(
                        Dd[:], wine, _tview(do_t, 0, d_flat), AluOpType.mult)

                    # E (real path) and Mm (imag path) share one tile so
                    # the j-tree and final reduce process both paths per op.
                    EM = wp.tile([P, 2 * M], BF16, tag="EM", bufs=3)
                    nc.vector.tensor_tensor(
                        _tview(EM, 0, [(1, M)]), A[:], B[:], AluOpType.add)
                    nc.vector.tensor_tensor(
                        _tview(EM, M, [(1, M)]), Cc[:], Dd[:],
                        AluOpType.subtract)

                    # j-tree: fold the 5 taps into 2 stacked halves with
                    # contiguous adds (both paths per op), then one 2-slot
                    # strided reduce into the interleaved o_t.
                    VF = UC * NDF

                    def js2(j):
                        return _tview(
                            EM, j * NDF, [(M, 2), (JF, UC), (1, NDF)]
                        )

                    Z2 = wp.tile([P, 4 * VF], BF16, tag="Z2", bufs=3)
                    zt2 = wp.tile([P, 2 * VF], BF16, tag="zt2")
                    nc.vector.tensor_tensor(
                        _tview(Z2, 0, [(2 * VF, 2), (NDF, UC), (1, NDF)]),
                        js2(0), js2(1), AluOpType.add)
                    nc.vector.tensor_tensor(
                        _tview(zt2, 0, [(VF, 2), (NDF, UC), (1, NDF)]),
                        js2(2), js2(3), AluOpType.add)
                    nc.vector.tensor_tensor(
                        _tview(Z2, VF, [(2 * VF, 2), (NDF, UC), (1, NDF)]),
                        _tview(zt2, 0, [(VF, 2), (NDF, UC), (1, NDF)]),
                        js2(4), AluOpType.add)
                    nc.vector.tensor_reduce(
                        out=_tview(o_t, 0, [(1, 2), (C, UC), (2, NDF)]),
                        in_=_tview(
                            Z2, 0,
                            [(2 * VF, 2), (NDF, UC), (1, NDF), (VF, 2)],
                        ),
                        axis=mybir.AxisListType.X,
                        op=AluOpType.add,
                    )

                    nc.scalar.dma_start(
                        out=_view(
                            O, (base + uc0) * W, [(U * W, P), (W, UC), (1, C)]
                        ),
                        in_=_tview(o_t, 0, [(C, UC), (1, C)]),
                    )

                rows_per = N // PASS_SPLIT
                for ps in range(PASS_SPLIT):
                    r0 = base + ps * rows_per
                    nc.gpsimd.dma_start(
                        out=_view(O, r0 * W + C, [(W, rows_per), (1, PW)]),
                        in_=_view(
                            S32, (r0 + 2) * PW, [(PW, rows_per), (1, PW)]
                        ),
                    )

    nc.compile()
    return nc


def get_nc():
    if "nc" not in _NC_CACHE:
        _NC_CACHE["nc"] = _build_nc()
    return _NC_CACHE["nc"]


def prepare_inputs(spec, coefs, alpha):
    """Host-side shard prep. Returns in_maps for the 8 cores."""
    import ml_dtypes

    bf16 = ml_dtypes.bfloat16
    spec = np.ascontiguousarray(spec, dtype=np.float32)
    coefs = np.ascontiguousarray(coefs, dtype=np.float32)
    alpha = np.ascontiguousarray(alpha, dtype=np.float32)
    T = spec.shape[0]
    assert T == T_FULL

    h_rows = (N_CORES - 1) * TC + TC_PAD + 4
    # swapped-halo DF planes (bf16) and passthrough plane (f32)
    HE = np.zeros((h_rows, NDF), bf16)
    HO = np.zeros((h_rows, NDF), bf16)
    HP = np.zeros((h_rows, PW), np.float32)
    sw = np.arange(T)
    sw[0], sw[1] = 1, 0
    HE[2 : T + 2] = spec[sw, :NDF, 0].astype(bf16)
    HO[2 : T + 2] = spec[sw, :NDF, 1].astype(bf16)
    HP[2 : T + 2] = spec[sw, NDF:, :].reshape(T, PW)

    d_rows = (N_CORES - 1) * TC + TC_PAD
    a = np.ascontiguousarray(alpha, dtype=np.float32)[:, 0, None, None]
    DEv = np.zeros((d_rows, ORDER, NDF), np.float32)
    DOv = np.zeros((d_rows, ORDER, NDF), np.float32)
    np.multiply(a, coefs[..., 0], out=DEv[:T])
    np.multiply(-a, coefs[..., 1], out=DOv[:T])
    DEv[:T, 2, :] += (1.0 - a[:, 0, 0])[:, None]  # base tap: win[t,2] = H[t+2]
    DEv = DEv.reshape(d_rows, JF).astype(bf16)
    DOv = DOv.reshape(d_rows, JF).astype(bf16)

    in_maps = [
        {
            "se": HE[c * TC : c * TC + TC_PAD + 4],
            "so": HO[c * TC : c * TC + TC_PAD + 4],
            "s32": HP[c * TC : c * TC + TC_PAD + 4],
            "de": DEv[c * TC : c * TC + TC_PAD],
            "do": DOv[c * TC : c * TC + TC_PAD],
        }
        for c in range(N_CORES)
    ]
    return in_maps


def run_spmd(in_maps, trace=False, **kwargs):
    from concourse.bass_utils import run_bass_kernel_spmd

    nc = get_nc()
    return run_bass_kernel_spmd(
        nc, in_maps, list(range(N_CORES)), trace=trace, **kwargs
    )


def kernel(spec, coefs, alpha):
    in_maps = prepare_inputs(spec, coefs, alpha)
    res = run_spmd(in_maps).results
    out = np.concatenate([r["o"][:TC] for r in res], axis=0)
    return out.reshape(T_FULL, NFREQ, 2)
